# revision 1
# baseline (speedup 1.0000x reference)
"""nn_CoupFourGAT — Trainium2 Bass kernel.

Host (numpy): STFT framing + rFFT + map_w projection + 3x3 conv + QKV
projections + gate-fusion MLP + decoder (all tiny, ~100 MFLOP total).
Device (8 NeuronCores, SPMD): the dominant masked-softmax GAT attention:
per (batch, re/im, head) triple computes
  E^T = adj^T * exp(scale * K Q^T);  raw = [E @ V | E @ 1]
with the softmax denominator fused in as an extra ones-column of V.
Data-parallel over batch: core c handles batches [16c, 16c+16).

Device layout (per core, NTRI=128 triples):
 - 4 triples packed per 128-partition tile at partition offsets
   {0,32,64,96} (PE tile_position alignment), 8 such slots of 256 cols
   per DMA tile -> 4 qt + 4 kt DMAs for all 128 triples.
 - matmul1 in bf16 (1 cyc/row): E^T chunks (m 0:128, m 119:247) into one
   2-bank PSUM tile per 2 triples; single exp (ACT, bf16 out) + single
   adj-mask multiply (DVE, bf16 4x) per 2 triples.
 - matmul2 (E@Vp) accumulates 8 triples per PSUM bank (col slots);
   gpsimd drains PSUM->SBUF; 8 output DMAs total.
"""
import math
import os
import numpy as np
import ml_dtypes

B, N, L, H = 128, 247, 12, 4
NFFT, HOP, FRAMES = 256, 246, 13
LAM = 0.01
NCORES = 8
BLOC = B // NCORES          # 16 batches per core
NTRI = BLOC * 2 * H         # 128 (batch, re/im, head) triples per core
NBIG = 4                    # output big groups per core
TPB = NTRI // NBIG          # 32 triples per output big group
SLOT = 256                  # col block per triple slot in qt/kt tiles
NSLOT = (NTRI + 2) // 3     # 43 slots of 3 triples (bases {0,32,64})
SPT = 8                     # slots per qt/kt tile
NQKT = (NSLOT + SPT - 1) // SPT  # 6 qt/kt tiles
SCALE = 1.0 / math.sqrt(L)
BF16 = ml_dtypes.bfloat16

_DEVICE_CACHE = {}
LAST_EXEC_NS = None


def _erf(x):
    try:
        from scipy.special import erf
        return erf(x)
    except Exception:
        return np.vectorize(math.erf, otypes=[np.float64])(x)


def _ln(t, g, b, eps=1e-5):
    m = t.mean(-1, keepdims=True)
    v = ((t - m) ** 2).mean(-1, keepdims=True)
    return (t - m) / np.sqrt(v + eps) * g + b


def _softshrink(t):
    return np.where(t > LAM, t - LAM, np.where(t < -LAM, t + LAM, 0.0))


def _leaky(t):
    return np.where(t >= 0, t, 0.01 * t)


def _front(x, map_w, map_b, conv_w, conv_b):
    """x (B,N,L) -> tr, ti (B,N,L): FFT + map + conv + leaky + residual."""
    Bc = x.shape[0]
    xf = x.reshape(Bc, -1)
    pad = np.pad(xf, ((0, 0), (NFFT // 2, NFFT // 2)), mode='reflect')
    idx = np.arange(FRAMES)[:, None] * HOP + np.arange(NFFT)[None, :]
    frames = pad[:, idx]                               # (B, 13, 256)
    spec = np.fft.rfft(frames.astype(np.float64), axis=-1)  # (B,13,129)
    spec = np.swapaxes(spec, 1, 2)[:, :, :L]           # (B, 129, 12)
    xr = spec.real.reshape(Bc, L, 129) @ map_w.T.astype(np.float64) + map_b
    xi = spec.imag.reshape(Bc, L, 129) @ map_w.T.astype(np.float64) + map_b
    xr = xr.reshape(Bc, N, L).astype(np.float32)
    xi = xi.reshape(Bc, N, L).astype(np.float32)
    vec = np.stack([xr, xi], axis=-1)                  # (B, N, L, 2)
    v2 = vec.reshape(Bc, 2, N, L)
    vp = np.pad(v2, ((0, 0), (0, 0), (1, 1), (1, 1)))
    c = np.zeros_like(v2)
    for o in range(2):
        for i in range(2):
            for ky in range(3):
                for kx in range(3):
                    c[:, o] += conv_w[o, i, ky, kx] * vp[:, i, ky:ky + N, kx:kx + L]
    c = c + conv_b[None, :, None, None]
    c = _leaky(c.reshape(Bc, N, L, 2)) + vec
    return c[..., 0], c[..., 1]


def _pack_host(qt, kt, vp, adj):
    """qt/kt: (NC, NTRI, L, N) f32; vp: (NC, NTRI, N, 13) f32; adj (N, N).

    Returns per-core device arrays:
      qt_h/kt_h (NC, NBIG, 128, 8*SLOT) bf16  [4 triples x 32 partitions,
                                               8 slots x 256 cols]
      vpa_h/vpb_h (NC, 128, NTRI*13) bf16     [m-chunks 0:128 / 119:247,
                                               overlap rows zeroed in B]
      adjm_h (128, 1024) bf16                 [mask, chunk-packed, x2]
    """
    NC = NCORES

    def qk_pack(a):
        out = np.zeros((NC, NQKT, 3, 32, SPT, SLOT), BF16)
        # t = 3*(SPT*tile + s) + j ; partition 32*j + i ; col SLOT*s + n
        pad = np.zeros((NC, NQKT * SPT * 3, L, N), a.dtype)
        pad[:, :NTRI] = a
        src = pad.reshape(NC, NQKT, SPT, 3, L, N).transpose(0, 1, 3, 4, 2, 5)
        out[:, :, :, :L, :, :N] = src.astype(BF16)
        return np.ascontiguousarray(out.reshape(NC, NQKT, 96, SPT * SLOT))

    qk_h = np.concatenate([qk_pack(qt), qk_pack(kt)], axis=3)

    vpa = vp[:, :, :128, :]                                  # (NC,NTRI,128,13)
    vpb = vp[:, :, 119:, :].copy()
    vpb[:, :, :9, :] = 0.0                                   # kill overlap
    vpa_h = vpa.transpose(0, 2, 1, 3).reshape(NC, 128, NTRI * 13).astype(BF16)
    vpb_h = vpb.transpose(0, 2, 1, 3).reshape(NC, 128, NTRI * 13).astype(BF16)

    adjT = adj.T.astype(np.float32)                          # adjT[m,n]
    adjm = np.zeros((128, 512), np.float32)
    adjm[:, :N] = adjT[:128, :]
    adjm[:, 256:256 + N] = adjT[119:, :]
    adjm_h = np.concatenate([adjm, adjm], axis=1).astype(BF16)  # (128, 1024)
    # single consts tensor: [vpa | vpb | adjm] along free dim
    consts_h = np.ascontiguousarray(
        np.concatenate([vpa_h, vpb_h, adjm_h[None].repeat(NCORES, 0)], axis=2))
    return qk_h, consts_h


def _unpack_raw(rawA, rawB):
    """rawA/rawB (NC, NBIG, 128, 416) f32 -> raw (NC, NTRI, N, 13)."""
    NC = NCORES
    rA = rawA.reshape(NC, NBIG, 128, TPB, 13).transpose(0, 1, 3, 2, 4)
    rA = rA.reshape(NC, NTRI, 128, 13)
    rB = rawB.reshape(NC, NBIG, 128, TPB, 13).transpose(0, 1, 3, 2, 4)
    rB = rB.reshape(NC, NTRI, 128, 13)[:, :, :119]
    return np.concatenate([rA, rB], axis=2)


def _device_model_numpy(qk_h, consts_h):
    """Numpy mirror of the device program (per core), on the packed arrays."""
    f32 = np.float32
    QK = SPT * SLOT
    vpa_h = consts_h[:, :NTRI * 13]
    vpb_h = consts_h[:, NTRI * 13:2 * NTRI * 13]
    adjm = consts_h[:, 2 * NTRI * 13:].astype(f32)
    rawA = np.zeros((NBIG, 128, TPB * 13), f32)
    rawB = np.zeros((NBIG, 128, TPB * 13), f32)
    poA = poB = None
    for u in range(NTRI // 2):
        ps = np.empty((128, 1024), f32)
        if u % 4 == 0:
            poA = np.zeros((128, 104), f32)
            poB = np.zeros((128, 104), f32)
        for hh in range(2):
            t = 2 * u + hh
            sg, j = divmod(t, 3)
            ti, s_ = divmod(sg, SPT)
            qs = qk_h[ti][32 * j:32 * j + L,
                          SLOT * s_:SLOT * s_ + 256].astype(f32)
            ka = qk_h[ti][32 * j:32 * j + L,
                          QK + SLOT * s_:QK + SLOT * s_ + 128].astype(f32)
            kb = qk_h[ti][32 * j:32 * j + L,
                          QK + SLOT * s_ + 119:QK + SLOT * s_ + 247].astype(f32)
            co = 512 * hh
            ps[:, co:co + 256] = ka.T @ qs
            ps[:, co + 256:co + 512] = kb.T @ qs
        e = np.exp(SCALE * ps).astype(BF16).astype(f32)
        em = (e * adjm).astype(BF16).astype(f32)
        for hh in range(2):
            t = 2 * u + hh
            co = 512 * hh
            pcol = 13 * (t % 8)
            va = vpa_h[:, 13 * t:13 * t + 13].astype(f32)
            vb = vpb_h[:, 13 * t:13 * t + 13].astype(f32)
            poA[:, pcol:pcol + 13] = (em[:, co:co + 128].T @ va
                                      + em[:, co + 256:co + 384].T @ vb)
            poB[:119, pcol:pcol + 13] = (em[:, co + 128:co + 247].T @ va
                                         + em[:, co + 384:co + 503].T @ vb)
        if u % 4 == 3:
            t = 2 * u
            big, og = t // TPB, (t % TPB) // 8
            rawA[big, :, 104 * og:104 * og + 104] = poA
            rawB[big, :, 104 * og:104 * og + 104] = poB
    return rawA, rawB


def _prune_redundant_waits(nc):
    """Drop sync waits transitively implied by another wait on the same
    instruction: if wait w2's producing instruction itself waited on
    semaphore w.sem >= w.value, then w2 being satisfied implies w is too
    (sem values are monotonic).  Needed because walrus's HW-decoded PE
    matmul struct has a single sync-wait slot, and the tile scheduler
    emits (ACT, PE) wait pairs on PSUM-reuse matmuls where the PE wait
    is implied by the ACT one."""
    from collections import defaultdict
    for fn in nc.m.functions:
        for blk in fn.blocks:
            insts = list(blk.instructions)
            prod = defaultdict(list)
            for inst in insts:
                si = inst.sync_info
                if si is None:
                    continue
                for up in (si.on_update or []):
                    cum = (prod[up.ant_name][-1][0] if prod[up.ant_name]
                           else 0) + (up.update_value or 1)
                    prod[up.ant_name].append((cum, inst))

            def covers(w2, w):
                # True if waiting on w2 guarantees w is already satisfied.
                for cum, p in prod.get(w2.ant_name, []):
                    if cum >= w2.wait_value:
                        psi = p.sync_info
                        for pw in (psi.on_wait or []) if psi else []:
                            if (pw.ant_name == w.ant_name
                                    and pw.wait_value >= w.wait_value):
                                return True
                        return False
                return False

            for inst in insts:
                si = inst.sync_info
                if si is None or not si.on_wait or len(si.on_wait) < 2:
                    continue
                keep = list(si.on_wait)
                changed = True
                while changed and len(keep) > 1:
                    changed = False
                    for w in list(keep):
                        others = [x for x in keep if x is not w]
                        if any(covers(w2, w) for w2 in others):
                            keep.remove(w)
                            changed = True
                            break
                if len(keep) < len(si.on_wait):
                    si.on_wait = keep


def _build_device():
    import concourse.bass as bass
    import concourse.mybir as mybir
    from concourse import bacc, tile

    f32 = mybir.dt.float32
    bf = mybir.dt.bfloat16
    nc = bacc.Bacc("TRN2", target_bir_lowering=False)
    qk_d = nc.declare_dram_parameter("qk", [NQKT, 96, 2 * SPT * SLOT], bf, isOutput=False)
    consts_d = nc.declare_dram_parameter("consts", [128, 2 * NTRI * 13 + 1024], bf, isOutput=False)
    rawA_d = nc.declare_dram_parameter("rawA", [NBIG, 128, TPB * 13], f32, isOutput=True)
    rawB_d = nc.declare_dram_parameter("rawB", [NBIG, 128, TPB * 13], f32, isOutput=True)

    EXP = mybir.ActivationFunctionType.Exp
    with tile.TileContext(nc) as tc:
        with (
            tc.tile_pool(name="const", bufs=1) as cpool,
            tc.tile_pool(name="work", bufs=3) as work,
            tc.tile_pool(name="ostage", bufs=2) as opool,
            tc.tile_pool(name="psum", bufs=2, space=bass.MemorySpace.PSUM) as psum,
        ):
            # consts (vp + adj mask) first: unit 0's mask-mul and matmul2
            # block on it; the bulk qk tiles (needed progressively) follow.
            consts = cpool.tile([128, 2 * NTRI * 13 + 1024], bf, tag="consts")
            nc.sync.dma_start(out=consts[:], in_=consts_d[:, :])
            vpa = consts[:, :NTRI * 13]
            vpb = consts[:, NTRI * 13:2 * NTRI * 13]
            adjm = consts[:, 2 * NTRI * 13:]
            QK = SPT * SLOT
            qk_tiles = {}
            for ti in range(NQKT):
                qk_s = cpool.tile([96, 2 * SPT * SLOT], bf, tag=f"qk{ti}")
                nc.sync.dma_start(out=qk_s[:], in_=qk_d[ti])
                qk_tiles[ti] = qk_s

            po = oA = oB = None
            for u in range(NTRI // 2):
                if u % (TPB // 2) == 0:
                    oA = opool.tile([128, TPB * 13], f32, tag="oA")
                    oB = opool.tile([128, TPB * 13], f32, tag="oB")
                if u % 4 == 0:
                    po = psum.tile([128, 512], f32, tag="po")
                ps = psum.tile([128, 1024], f32, tag="ps")
                for hh in range(2):
                    t = 2 * u + hh
                    sg, j = divmod(t, 3)
                    ti, s_ = divmod(sg, SPT)
                    qk_s = qk_tiles[ti]
                    qs = qk_s[32 * j:32 * j + L, SLOT * s_:SLOT * s_ + 256]
                    co = 512 * hh
                    nc.tensor.matmul(
                        ps[:, co:co + 256],
                        qk_s[32 * j:32 * j + L, QK + SLOT * s_:QK + SLOT * s_ + 128],
                        qs, start=True, stop=True)
                    nc.tensor.matmul(
                        ps[:, co + 256:co + 512],
                        qk_s[32 * j:32 * j + L,
                             QK + SLOT * s_ + 119:QK + SLOT * s_ + 247],
                        qs, start=True, stop=True)
                e = work.tile([128, 1024], bf, tag="e")
                nc.scalar.activation(e[:], ps[:], EXP, scale=SCALE)
                em = work.tile([128, 1024], bf, tag="em")
                nc.vector.tensor_mul(em[:], e[:], adjm)
                for hh in range(2):
                    t = 2 * u + hh
                    co = 512 * hh
                    pcol = 13 * (t % 8)
                    va = vpa[:, 13 * t:13 * t + 13]
                    vb = vpb[:, 13 * t:13 * t + 13]
                    nc.tensor.matmul(po[:, pcol:pcol + 13],
                                     em[:, co:co + 128], va,
                                     start=True, stop=False)
                    nc.tensor.matmul(po[:, pcol:pcol + 13],
                                     em[:, co + 256:co + 384], vb,
                                     start=False, stop=True)
                    nc.tensor.matmul(po[:119, 256 + pcol:256 + pcol + 13],
                                     em[:, co + 128:co + 247], va,
                                     start=True, stop=False)
                    nc.tensor.matmul(po[:119, 256 + pcol:256 + pcol + 13],
                                     em[:, co + 384:co + 503], vb,
                                     start=False, stop=True)
                if u % 4 == 3:
                    t = 2 * u
                    big, og = t // TPB, (t % TPB) // 8
                    nc.vector.tensor_copy(oA[:, 104 * og:104 * og + 104],
                                          po[:, :104])
                    nc.vector.tensor_copy(oB[:119, 104 * og:104 * og + 104],
                                          po[:119, 256:360])
                    if og == 3:
                        nc.sync.dma_start(out=rawA_d[big], in_=oA[:])
                        nc.sync.dma_start(out=rawB_d[big], in_=oB[:])
    _prune_redundant_waits(nc)
    nc.finalize()
    return nc


def _attention_device(qk_h, consts_h):
    global LAST_EXEC_NS
    from concourse.bass_utils import run_bass_kernel_spmd
    if "nc" not in _DEVICE_CACHE:
        _DEVICE_CACHE["nc"] = _build_device()
    nc = _DEVICE_CACHE["nc"]
    in_maps = []
    for c in range(NCORES):
        in_maps.append({"qk": qk_h[c], "consts": consts_h[c]})
    trace = bool(os.environ.get("KERNEL_TRACE"))
    res = run_bass_kernel_spmd(nc, in_maps, list(range(NCORES)), trace=trace)
    if trace:
        LAST_EXEC_NS = res.exec_time_ns
        _DEVICE_CACHE["last_results"] = res
    rawA = np.stack([res.results[c]["rawA"] for c in range(NCORES)])
    rawB = np.stack([res.results[c]["rawB"] for c in range(NCORES)])
    return rawA, rawB


def kernel(x, prc, adj, Wq, Wk, Wv, ln_g, ln_b, ln2_g, ln2_b, enc_w, enc_b,
           dec_w, dec_b, map_w, map_b, conv_w, conv_b, g1_w, g1_b,
           gln_g, gln_b, g2_w, g2_b):
    x = np.asarray(x, np.float32)
    prc = np.asarray(prc, np.float32)
    tr, ti = _front(x, map_w, map_b, conv_w, conv_b)

    # QKV projections for both re/im streams: t (B,N,L) @ W[h] (L,L)
    ts = np.stack([tr, ti], axis=1)                    # (B, 2, N, L)
    Q = np.einsum('brnl,hlo->brhon', ts, Wq)           # (B,2,H,L,N) = Q^T
    K = np.einsum('brnl,hlo->brhon', ts, Wk)
    V = np.einsum('brnl,hlo->brhno', ts, Wv)           # (B,2,H,N,L)
    ones = np.ones((B, 2, H, N, 1), np.float32)
    Vp = np.concatenate([V, ones], axis=-1)            # (B,2,H,N,L+1)

    qt = Q.reshape(NCORES, NTRI, L, N).astype(np.float32)
    kt = K.reshape(NCORES, NTRI, L, N).astype(np.float32)
    vp = Vp.reshape(NCORES, NTRI, N, L + 1).astype(np.float32)
    qk_h, consts_h = _pack_host(qt, kt, vp, adj)

    if os.environ.get("KERNEL_NUMPY"):
        outs = [_device_model_numpy(qk_h[c], consts_h[c]) for c in range(NCORES)]
        rawA = np.stack([o[0] for o in outs])
        rawB = np.stack([o[1] for o in outs])
    else:
        try:
            rawA, rawB = _attention_device(qk_h, consts_h)
        except Exception as e:
            import traceback
            traceback.print_exc()
            print(f"DEVICE PATH FAILED ({e}); falling back to numpy")
            outs = [_device_model_numpy(qk_h[c], consts_h[c])
                    for c in range(NCORES)]
            rawA = np.stack([o[0] for o in outs])
            rawB = np.stack([o[1] for o in outs])

    raw = _unpack_raw(rawA, rawB)                      # (NC, NTRI, N, 13)
    raw = raw.reshape(B, 2, H, N, L + 1)
    out_av = raw[..., :L] / raw[..., L:L + 1]          # (B,2,H,N,L)
    out_ln = _ln(out_av, ln_g, ln_b)                   # post-attention LN

    res = []
    for ri in range(2):
        out = np.transpose(out_ln[:, ri], (1, 0, 2, 3))  # (H,B,N,L)
        hs = np.transpose(out, (1, 0, 2, 3))             # (B,H,N,L)
        nf = np.moveaxis(hs, 2, 0)                       # (N,B,H,L)
        nfr = nf.reshape(N, H, B, L)
        avg = nfr.mean(axis=1)                           # (N,B,L)
        mx = nfr.max(axis=1)
        z = np.concatenate([avg, mx], axis=-1) @ g1_w.T + g1_b
        z = _ln(z, gln_g, gln_b)
        z = z * 0.5 * (1.0 + _erf(z / math.sqrt(2.0)))   # exact gelu
        z = 1.0 / (1.0 + np.exp(-(z @ g2_w.T + g2_b)))
        fused = z * avg + (1.0 - z) * mx + nf.mean(axis=2)
        res.append(np.transpose(fused, (1, 0, 2)))       # (B,N,L)

    xr = _softshrink(res[0])
    xi = _softshrink(res[1])
    f = xr * enc_w[0, 0] + xi * enc_w[0, 1] + enc_b[0]
    f = _ln(f, ln2_g, ln2_b) + x
    e = f * enc_w[0, 0] + prc * enc_w[0, 1] + enc_b[0]
    a = _leaky(e)
    out = a @ dec_w.T + dec_b + x
    return out.astype(np.float32)

